# revision 8
# baseline (speedup 1.0000x reference)
"""VQ codebook kernel for Trainium2, data-parallel over 8 NeuronCores.

Problem (hardcoded shapes): z (16, 512, 64, 64) f32, codebook (1024, 512) f32.
Returns (z_q, q_loss, distance_prob) matching the reference:
    z_flat = z.transpose(0,2,3,1).reshape(-1, 512)          # (65536, 512)
    dist   = ||z||^2 + ||c||^2 - 2 z @ c.T                  # (65536, 1024)
    min_idx = argmin(dist, axis=1)  (first index on fp32 ties)
    distance_prob = softmax(-dist / 0.5, axis=1)
    z_q = z_flat + (codebook[min_idx] - z_flat)   # straight-through, fp32
    q_loss = 1.25 * mean((codebook[min_idx] - z_flat)^2)

Sharding: data-parallel on z rows — core i handles b in {2i, 2i+1} (8192 rows),
codebook replicated.  Scalar loss reduced on host.

Numerics: the argmin must replicate the reference's fp32 rounding of
dist = fl(fl(zn + cn) - 2M) (values ~512, fp32 ulp 6.1e-5) including
first-index tie-breaks, so M needs ~1e-7 accuracy.  The matmul runs as three
fp16 matmuls (Dekker-split z = zh + zl, C2 = 2^10*cb.T = Ch + Cl; the PE
preserves fp16 subnormals) accumulating zh*Ch + zh*Cl + zl*Ch in fp32 PSUM,
giving psum = 2^9 * 2M with ~1e-8 error.  The whole dist pipeline is carried
at 2^9 scale (exact power-of-two) and descaled only in exp (scale=2^-8) and
in the host loss reduction.  Row norms zn may be computed in any fp32 order:
a per-row constant shift moves the whole row rigidly on the fp32 grid within
a binade, preserving the quantized comparison structure.
"""

import sys

for _p in ("/opt/trn_rl_repo", "/root/.axon_site/_ro/trn_rl_repo"):
    if _p not in sys.path:
        sys.path.append(_p)

import numpy as np

import concourse.bass as bass
import concourse.tile as tile
from concourse import mybir
from concourse.bass_utils import run_bass_kernel_spmd
from concourse.masks import make_identity

P = 128
D = 512
K = 1024
B, H, W = 16, 64, 64
N_CORES = 8
B_PER_CORE = B // N_CORES            # 2
ROWS_PER_CORE = B_PER_CORE * H * W   # 8192
N_SUB = ROWS_PER_CORE // P           # 64 subtiles of 128 rows
SUB_PER_BLK = 4                      # 512 rows per z-load block
N_BLK = N_SUB // SUB_PER_BLK         # 16

F32 = mybir.dt.float32
F16 = mybir.dt.float16
U32 = mybir.dt.uint32


def _split_waits(nc, limit=1):
    """This walrus build accepts at most one sync-wait per instruction; move
    extra waits onto preceding same-engine NOPs."""
    for fn in nc.m.functions:
        for blk in fn.blocks:
            new_insts = []
            for inst in blk.instructions:
                si = inst.sync_info
                if si is not None and si.on_wait and len(si.on_wait) > limit:
                    waits = list(si.on_wait)
                    extra, keep = waits[:-limit], waits[-limit:]
                    while extra:
                        chunk, extra = extra[:limit], extra[limit:]
                        nop = mybir.InstNoOp(
                            name=nc.get_next_instruction_name(),
                            engine=inst.engine,
                            ins=[],
                            outs=[],
                            sync_info=mybir.SyncInfo(on_wait=chunk, on_update=[]),
                        )
                        nc.register_instruction(nop)
                        new_insts.append(nop)
                    inst.sync_info = mybir.SyncInfo(
                        on_wait=keep, on_update=list(si.on_update or [])
                    )
                new_insts.append(inst)
            blk.instructions[:] = new_insts


def _build():
    nc = bass.Bass(num_swdge_queues=4)

    zh_in = nc.declare_dram_parameter("zh", [B_PER_CORE, D, H * W], F16, isOutput=False)
    zl_in = nc.declare_dram_parameter("zl", [B_PER_CORE, D, H * W], F16, isOutput=False)
    zc = nc.declare_dram_parameter("zc", [B_PER_CORE, D, H * W], F32, isOutput=False)
    cbh_in = nc.declare_dram_parameter("cbh", [D, K], F16, isOutput=False)
    cbl_in = nc.declare_dram_parameter("cbl", [D, K], F16, isOutput=False)
    cnp = nc.declare_dram_parameter("cn2", [1, K], F32, isOutput=False)  # 2^9*cn
    znr = nc.declare_dram_parameter("znr2", [P, N_SUB], F32, isOutput=False)  # 2^9*zn
    cbg = nc.declare_dram_parameter("cbg", [K, D], F32, isOutput=False)  # gather table

    probs_o = nc.declare_dram_parameter("probs", [ROWS_PER_CORE, K], F32, isOutput=True)
    zq_o = nc.declare_dram_parameter("zq", [B_PER_CORE, D, H * W], F32, isOutput=True)
    m_o = nc.declare_dram_parameter("mrow", [P, N_SUB], F32, isOutput=True)
    idx_o = nc.declare_dram_parameter("idxr", [P, N_SUB], U32, isOutput=True)

    with tile.TileContext(nc) as tc:
        with (
            tc.tile_pool(name="statics", bufs=1) as statics,
            tc.tile_pool(name="zpool", bufs=3) as zpool,
            tc.tile_pool(name="tpool", bufs=3) as tpool,
            tc.tile_pool(name="ndpool", bufs=3) as ndpool,
            tc.tile_pool(name="epool", bufs=3) as epool,
            tc.tile_pool(name="prpool", bufs=3) as prpool,
            tc.tile_pool(name="gqpool", bufs=3) as gqpool,
            tc.tile_pool(name="zqpool", bufs=3) as zqpool,
            tc.tile_pool(name="small", bufs=8) as small,
            tc.tile_pool(name="pspool", bufs=2, space="PSUM") as pspool,
            tc.tile_pool(name="tppool", bufs=2, space="PSUM") as tppool,
        ):
            # resident tensors
            cbh = statics.tile([P, D // P, K], F16)
            nc.sync.dma_start(
                out=cbh[:], in_=cbh_in[:].rearrange("(do di) k -> di do k", di=P)
            )
            cbl = statics.tile([P, D // P, K], F16)
            nc.sync.dma_start(
                out=cbl[:], in_=cbl_in[:].rearrange("(do di) k -> di do k", di=P)
            )
            cn_sb = statics.tile([P, K], F32)
            nc.sync.dma_start(out=cn_sb[:], in_=cnp[:].to_broadcast([P, K]))
            znr_sb = statics.tile([P, N_SUB], F32)
            nc.sync.dma_start(out=znr_sb[:], in_=znr[:])
            ident = statics.tile([P, P], F32)
            make_identity(nc, ident[:])
            msb = statics.tile([P, N_SUB], F32)
            isb = statics.tile([P, N_SUB], U32)

            def emit_zq_tail(gq, zf_s, bb, poff, sub):
                """Transpose gathered z_q to [d, pix], apply the straight-
                through fp32 rounding, and store.  Called one subtile late so
                the PE transposes slot in behind the next subtile's matmuls
                instead of stalling on the gather chain."""
                pst = tppool.tile([P, 512], F32)
                for k in range(D // P):
                    nc.tensor.transpose(
                        pst[:, k * P : (k + 1) * P],
                        gq[:, k * P : (k + 1) * P],
                        ident[:],
                    )
                d1 = zqpool.tile([P, D // P, P], F32, tag="d1")
                nc.vector.tensor_tensor(
                    d1[:],
                    pst[:].rearrange("p (do x) -> p do x", do=D // P),
                    zf_s,
                    mybir.AluOpType.subtract,
                )
                zqo = zqpool.tile([P, D // P, P], F32, tag="zqo")
                nc.vector.tensor_tensor(zqo[:], d1[:], zf_s, mybir.AluOpType.add)
                nc.sync.dma_start(
                    out=zq_o[bb].rearrange("(do di) x -> di do x", di=P)[
                        :, :, poff + sub * P : poff + (sub + 1) * P
                    ],
                    in_=zqo[:],
                )

            pending = None
            for blk in range(N_BLK):
                bb = blk // (N_BLK // B_PER_CORE)
                poff = (blk * SUB_PER_BLK * P) % (H * W)
                zspan = SUB_PER_BLK * P
                zht = zpool.tile([P, D // P, zspan], F16, tag="zh")
                zlt = zpool.tile([P, D // P, zspan], F16, tag="zl")
                zft = zpool.tile([P, D // P, zspan], F32, tag="zf")
                for t_, src in ((zht, zh_in), (zlt, zl_in), (zft, zc)):
                    nc.sync.dma_start(
                        out=t_[:],
                        in_=src[bb].rearrange("(do di) x -> di do x", di=P)[
                            :, :, poff : poff + zspan
                        ],
                    )
                for sub in range(SUB_PER_BLK):
                    st = blk * SUB_PER_BLK + sub
                    pxs = slice(sub * P, (sub + 1) * P)
                    ps = pspool.tile([P, K], F32)
                    for n in range(K // 512):
                        ns = slice(n * 512, (n + 1) * 512)
                        first = True
                        for k in range(D // P):
                            for zt_, cb_ in (
                                (zht, cbh),
                                (zht, cbl),
                                (zlt, cbh),
                            ):
                                nc.tensor.matmul(
                                    ps[:, ns],
                                    lhsT=zt_[:, k, pxs],
                                    rhs=cb_[:, k, ns],
                                    start=first,
                                    stop=(k == D // P - 1 and zt_ is zlt),
                                )
                                first = False
                    if pending is not None:
                        emit_zq_tail(*pending)
                        pending = None
                    # t2 = fl(2^9*cn + 2^9*zn) = 2^9 * fl(cn + zn)
                    t = tpool.tile([P, K], F32)
                    nc.scalar.activation(
                        t[:], cn_sb[:], mybir.ActivationFunctionType.Identity,
                        bias=znr_sb[:, st : st + 1], scale=1.0,
                    )
                    # negdist2 = fl(psum - t2) = 2^9 * (-dist)
                    nd = ndpool.tile([P, K], F32)
                    nc.vector.tensor_tensor(
                        nd[:], ps[:], t[:], mybir.AluOpType.subtract
                    )
                    # first-index argmax of negdist == argmin of dist
                    m8 = small.tile([P, 8], F32)
                    nc.vector.max(m8[:], nd[:])
                    idx8 = small.tile([P, 8], U32)
                    nc.vector.max_index(idx8[:], m8[:], nd[:])
                    # e = exp(2^-8*negdist2 - 2^-8*m2) = exp(-2*(dist-rowmin))
                    biasm = small.tile([P, 1], F32)
                    nc.scalar.mul(biasm[:], m8[:, :1], -0.00390625)  # -2^-8
                    e = epool.tile([P, K], F32)
                    s = small.tile([P, 1], F32)
                    nc.scalar.activation(
                        e[:], nd[:], mybir.ActivationFunctionType.Exp,
                        bias=biasm[:], scale=0.00390625, accum_out=s[:],
                    )
                    r = small.tile([P, 1], F32)
                    nc.vector.reciprocal(r[:], s[:])
                    pr = prpool.tile([P, K], F32)
                    nc.scalar.mul(pr[:], e[:], r[:])
                    nc.sync.dma_start(
                        out=probs_o[st * P : (st + 1) * P, :], in_=pr[:]
                    )
                    # z_q gather; transpose/STE/store deferred one subtile
                    gq = gqpool.tile([P, D], F32)
                    nc.gpsimd.indirect_dma_start(
                        out=gq[:], out_offset=None,
                        in_=cbg[:],
                        in_offset=bass.IndirectOffsetOnAxis(ap=idx8[:, :1], axis=0),
                    )
                    pending = (gq, zft[:, :, pxs], bb, poff, sub)
                    # stash per-row max(negdist2) and idx
                    nc.scalar.copy(msb[:, st : st + 1], m8[:, :1])
                    nc.vector.tensor_copy(isb[:, st : st + 1], idx8[:, :1])

            if pending is not None:
                emit_zq_tail(*pending)
            nc.sync.dma_start(out=m_o[:], in_=msb[:])
            nc.sync.dma_start(out=idx_o[:], in_=isb[:])

    _split_waits(nc, limit=1)
    return nc


_NC_CACHE = None


def _get_nc():
    global _NC_CACHE
    if _NC_CACHE is None:
        _NC_CACHE = _build()
    return _NC_CACHE


LAST_RES = None


def kernel(z, codebook, _want_timing=False):
    z = np.ascontiguousarray(z, dtype=np.float32)
    codebook = np.ascontiguousarray(codebook, dtype=np.float32)
    assert z.shape == (B, D, H, W) and codebook.shape == (K, D)

    C2 = np.ascontiguousarray(codebook.T) * np.float32(1024.0)   # (512,1024) exact 2^10
    cbh = C2.astype(np.float16)
    cbl = (C2 - cbh.astype(np.float32)).astype(np.float16)
    cn2 = (np.sum(codebook * codebook, axis=1, dtype=np.float32)
           * np.float32(512.0)).reshape(1, K)
    zn2 = (np.einsum("bdhw,bdhw->bhw", z, z, dtype=np.float32).astype(np.float32)
           * np.float32(512.0))

    zh = z.astype(np.float16)
    zl = (z - zh.astype(np.float32)).astype(np.float16)

    zc_view = z.reshape(B, D, H * W)
    zh_view = zh.reshape(B, D, H * W)
    zl_view = zl.reshape(B, D, H * W)
    in_maps = []
    for c in range(N_CORES):
        bsl = slice(c * B_PER_CORE, (c + 1) * B_PER_CORE)
        zn_c = zn2[bsl].reshape(-1)                                    # (8192,)
        znr2 = np.ascontiguousarray(zn_c.reshape(N_SUB, P).T)          # (128, 64)
        in_maps.append(
            dict(
                zh=np.ascontiguousarray(zh_view[bsl]),
                zl=np.ascontiguousarray(zl_view[bsl]),
                zc=np.ascontiguousarray(zc_view[bsl]),
                cbh=cbh,
                cbl=cbl,
                cn2=cn2,
                znr2=znr2,
                cbg=codebook,
            )
        )

    nc = _get_nc()
    res = run_bass_kernel_spmd(nc, in_maps, list(range(N_CORES)), trace=_want_timing)
    global LAST_RES
    LAST_RES = res

    probs = np.concatenate([r["probs"] for r in res.results], axis=0)  # (65536, 1024)
    zq = np.concatenate([r["zq"] for r in res.results], axis=0).reshape(B, D, H, W)

    # q_loss = 1.25 * mean((z_q - z_flat)^2); per-row squared distance equals
    # the fp32 dist at the argmin, which is -mrow * 2^-9.
    tot = 0.0
    for r in res.results:
        tot += -np.sum(r["mrow"].astype(np.float64))
    c_loss = tot / 512.0 / (B * H * W * D)
    q_loss = np.float32(1.25 * c_loss)

    return zq, q_loss, probs


# revision 10
# speedup vs baseline: 1.0002x; 1.0002x over previous
"""VQ codebook kernel for Trainium2, data-parallel over 8 NeuronCores.

Problem (hardcoded shapes): z (16, 512, 64, 64) f32, codebook (1024, 512) f32.
Returns (z_q, q_loss, distance_prob) matching the reference:
    z_flat = z.transpose(0,2,3,1).reshape(-1, 512)          # (65536, 512)
    dist   = ||z||^2 + ||c||^2 - 2 z @ c.T                  # (65536, 1024)
    min_idx = argmin(dist, axis=1)  (first index on fp32 ties)
    distance_prob = softmax(-dist / 0.5, axis=1)
    z_q = z_flat + (codebook[min_idx] - z_flat)   # straight-through, fp32
    q_loss = 1.25 * mean((codebook[min_idx] - z_flat)^2)

Sharding: data-parallel on z rows — core i handles b in {2i, 2i+1} (8192 rows),
codebook replicated.  Scalar loss reduced on host.

Numerics: the argmin must replicate the reference's fp32 rounding of
dist = fl(fl(zn + cn) - 2M) (values ~512, fp32 ulp 6.1e-5) including
first-index tie-breaks, so M needs ~1e-7 accuracy.  The matmul runs as three
fp16 matmuls (Dekker-split z = zh + zl, C2 = 2^10*cb.T = Ch + Cl; the PE
preserves fp16 subnormals) accumulating zh*Ch + zh*Cl + zl*Ch in fp32 PSUM,
giving psum = 2^9 * 2M with ~1e-8 error.  The whole dist pipeline is carried
at 2^9 scale (exact power-of-two) and descaled only in exp (scale=2^-8) and
in the host loss reduction.  Row norms zn may be computed in any fp32 order:
a per-row constant shift moves the whole row rigidly on the fp32 grid within
a binade, preserving the quantized comparison structure.
"""

import sys

for _p in ("/opt/trn_rl_repo", "/root/.axon_site/_ro/trn_rl_repo"):
    if _p not in sys.path:
        sys.path.append(_p)

import numpy as np

import concourse.bass as bass
import concourse.tile as tile
from concourse import mybir
from concourse.bass_utils import run_bass_kernel_spmd
from concourse.masks import make_identity

P = 128
D = 512
K = 1024
B, H, W = 16, 64, 64
N_CORES = 8
B_PER_CORE = B // N_CORES            # 2
ROWS_PER_CORE = B_PER_CORE * H * W   # 8192
N_SUB = ROWS_PER_CORE // P           # 64 subtiles of 128 rows
SUB_PER_BLK = 4                      # 512 rows per z-load block
N_BLK = N_SUB // SUB_PER_BLK         # 16

F32 = mybir.dt.float32
F16 = mybir.dt.float16
U32 = mybir.dt.uint32


def _split_waits(nc, limit=1):
    """This walrus build accepts at most one sync-wait per instruction; move
    extra waits onto preceding same-engine NOPs."""
    for fn in nc.m.functions:
        for blk in fn.blocks:
            new_insts = []
            for inst in blk.instructions:
                si = inst.sync_info
                if si is not None and si.on_wait and len(si.on_wait) > limit:
                    waits = list(si.on_wait)
                    extra, keep = waits[:-limit], waits[-limit:]
                    while extra:
                        chunk, extra = extra[:limit], extra[limit:]
                        nop = mybir.InstNoOp(
                            name=nc.get_next_instruction_name(),
                            engine=inst.engine,
                            ins=[],
                            outs=[],
                            sync_info=mybir.SyncInfo(on_wait=chunk, on_update=[]),
                        )
                        nc.register_instruction(nop)
                        new_insts.append(nop)
                    inst.sync_info = mybir.SyncInfo(
                        on_wait=keep, on_update=list(si.on_update or [])
                    )
                new_insts.append(inst)
            blk.instructions[:] = new_insts


def _build():
    nc = bass.Bass(num_swdge_queues=4)

    zh_in = nc.declare_dram_parameter("zh", [B_PER_CORE, D, H * W], F16, isOutput=False)
    zl_in = nc.declare_dram_parameter("zl", [B_PER_CORE, D, H * W], F16, isOutput=False)
    zc = nc.declare_dram_parameter("zc", [B_PER_CORE, D, H * W], F32, isOutput=False)
    cbh_in = nc.declare_dram_parameter("cbh", [D, K], F16, isOutput=False)
    cbl_in = nc.declare_dram_parameter("cbl", [D, K], F16, isOutput=False)
    cnp = nc.declare_dram_parameter("cn2", [1, K], F32, isOutput=False)  # 2^9*cn
    znr = nc.declare_dram_parameter("znr2", [P, N_SUB], F32, isOutput=False)  # 2^9*zn
    cbg = nc.declare_dram_parameter("cbg", [K, D], F32, isOutput=False)  # gather table

    probs_o = nc.declare_dram_parameter("probs", [ROWS_PER_CORE, K], F32, isOutput=True)
    zq_o = nc.declare_dram_parameter("zq", [B_PER_CORE, D, H * W], F32, isOutput=True)
    m_o = nc.declare_dram_parameter("mrow", [P, N_SUB], F32, isOutput=True)
    idx_o = nc.declare_dram_parameter("idxr", [P, N_SUB], U32, isOutput=True)

    with tile.TileContext(nc) as tc:
        with (
            tc.tile_pool(name="statics", bufs=1) as statics,
            tc.tile_pool(name="zpool", bufs=3) as zpool,
            tc.tile_pool(name="tpool", bufs=3) as tpool,
            tc.tile_pool(name="ndpool", bufs=3) as ndpool,
            tc.tile_pool(name="epool", bufs=3) as epool,
            tc.tile_pool(name="prpool", bufs=3) as prpool,
            tc.tile_pool(name="gqpool", bufs=3) as gqpool,
            tc.tile_pool(name="zqpool", bufs=3) as zqpool,
            tc.tile_pool(name="small", bufs=8) as small,
            tc.tile_pool(name="pspool", bufs=3, space="PSUM") as pspool,
            tc.tile_pool(name="tppool", bufs=2, space="PSUM") as tppool,
        ):
            # resident tensors
            cbh = statics.tile([P, D // P, K], F16)
            nc.sync.dma_start(
                out=cbh[:], in_=cbh_in[:].rearrange("(do di) k -> di do k", di=P)
            )
            cbl = statics.tile([P, D // P, K], F16)
            nc.sync.dma_start(
                out=cbl[:], in_=cbl_in[:].rearrange("(do di) k -> di do k", di=P)
            )
            cn_sb = statics.tile([P, K], F32)
            nc.sync.dma_start(out=cn_sb[:], in_=cnp[:].to_broadcast([P, K]))
            znr_sb = statics.tile([P, N_SUB], F32)
            nc.sync.dma_start(out=znr_sb[:], in_=znr[:])
            ident = statics.tile([P, P], F32)
            make_identity(nc, ident[:])
            msb = statics.tile([P, N_SUB], F32)
            isb = statics.tile([P, N_SUB], U32)

            def emit_zq_tail(gq, zf_s, bb, poff, sub):
                """Transpose gathered z_q to [d, pix], apply the straight-
                through fp32 rounding, and store.  Called one subtile late so
                the PE transposes slot in behind the next subtile's matmuls
                instead of stalling on the gather chain."""
                pst = tppool.tile([P, 512], F32)
                for k in range(D // P):
                    nc.tensor.transpose(
                        pst[:, k * P : (k + 1) * P],
                        gq[:, k * P : (k + 1) * P],
                        ident[:],
                    )
                d1 = zqpool.tile([P, D // P, P], F32, tag="d1")
                nc.vector.tensor_tensor(
                    d1[:],
                    pst[:].rearrange("p (do x) -> p do x", do=D // P),
                    zf_s,
                    mybir.AluOpType.subtract,
                )
                zqo = zqpool.tile([P, D // P, P], F32, tag="zqo")
                nc.vector.tensor_tensor(zqo[:], d1[:], zf_s, mybir.AluOpType.add)
                nc.sync.dma_start(
                    out=zq_o[bb].rearrange("(do di) x -> di do x", di=P)[
                        :, :, poff + sub * P : poff + (sub + 1) * P
                    ],
                    in_=zqo[:],
                )

            zspan = SUB_PER_BLK * P

            def load_z_block(blk):
                bb = blk // (N_BLK // B_PER_CORE)
                poff = (blk * SUB_PER_BLK * P) % (H * W)
                zht = zpool.tile([P, D // P, zspan], F16, tag="zh")
                zlt = zpool.tile([P, D // P, zspan], F16, tag="zl")
                zft = zpool.tile([P, D // P, zspan], F32, tag="zf")
                for t_, src in ((zht, zh_in), (zlt, zl_in), (zft, zc)):
                    nc.sync.dma_start(
                        out=t_[:],
                        in_=src[bb].rearrange("(do di) x -> di do x", di=P)[
                            :, :, poff : poff + zspan
                        ],
                    )
                return zht, zlt, zft

            pending = None
            zblocks = {0: load_z_block(0)}
            for blk in range(N_BLK):
                bb = blk // (N_BLK // B_PER_CORE)
                poff = (blk * SUB_PER_BLK * P) % (H * W)
                if blk + 1 < N_BLK:
                    zblocks[blk + 1] = load_z_block(blk + 1)
                zht, zlt, zft = zblocks.pop(blk)
                for sub in range(SUB_PER_BLK):
                    st = blk * SUB_PER_BLK + sub
                    pxs = slice(sub * P, (sub + 1) * P)
                    ps = pspool.tile([P, K], F32)
                    for n in range(K // 512):
                        ns = slice(n * 512, (n + 1) * 512)
                        first = True
                        for k in range(D // P):
                            for zt_, cb_ in (
                                (zht, cbh),
                                (zht, cbl),
                                (zlt, cbh),
                            ):
                                nc.tensor.matmul(
                                    ps[:, ns],
                                    lhsT=zt_[:, k, pxs],
                                    rhs=cb_[:, k, ns],
                                    start=first,
                                    stop=(k == D // P - 1 and zt_ is zlt),
                                )
                                first = False
                    if pending is not None:
                        emit_zq_tail(*pending)
                        pending = None
                    # t2 = fl(2^9*cn + 2^9*zn) = 2^9 * fl(cn + zn)
                    t = tpool.tile([P, K], F32)
                    nc.scalar.activation(
                        t[:], cn_sb[:], mybir.ActivationFunctionType.Identity,
                        bias=znr_sb[:, st : st + 1], scale=1.0,
                    )
                    # negdist2 = fl(psum - t2) = 2^9 * (-dist)
                    nd = ndpool.tile([P, K], F32)
                    nc.vector.tensor_tensor(
                        nd[:], ps[:], t[:], mybir.AluOpType.subtract
                    )
                    # first-index argmax of negdist == argmin of dist
                    m8 = small.tile([P, 8], F32)
                    nc.vector.max(m8[:], nd[:])
                    idx8 = small.tile([P, 8], U32)
                    nc.vector.max_index(idx8[:], m8[:], nd[:])
                    # e = exp(2^-8*negdist2 - 2^-8*m2) = exp(-2*(dist-rowmin))
                    biasm = small.tile([P, 1], F32)
                    nc.scalar.mul(biasm[:], m8[:, :1], -0.00390625)  # -2^-8
                    e = epool.tile([P, K], F32)
                    s = small.tile([P, 1], F32)
                    nc.scalar.activation(
                        e[:], nd[:], mybir.ActivationFunctionType.Exp,
                        bias=biasm[:], scale=0.00390625, accum_out=s[:],
                    )
                    r = small.tile([P, 1], F32)
                    nc.vector.reciprocal(r[:], s[:])
                    pr = prpool.tile([P, K], F32)
                    nc.scalar.mul(pr[:], e[:], r[:])
                    nc.sync.dma_start(
                        out=probs_o[st * P : (st + 1) * P, :], in_=pr[:]
                    )
                    # z_q gather; transpose/STE/store deferred one subtile
                    gq = gqpool.tile([P, D], F32)
                    nc.gpsimd.indirect_dma_start(
                        out=gq[:], out_offset=None,
                        in_=cbg[:],
                        in_offset=bass.IndirectOffsetOnAxis(ap=idx8[:, :1], axis=0),
                    )
                    pending = (gq, zft[:, :, pxs], bb, poff, sub)
                    # stash per-row max(negdist2) and idx
                    nc.scalar.copy(msb[:, st : st + 1], m8[:, :1])
                    nc.vector.tensor_copy(isb[:, st : st + 1], idx8[:, :1])

            if pending is not None:
                emit_zq_tail(*pending)
            nc.sync.dma_start(out=m_o[:], in_=msb[:])
            nc.sync.dma_start(out=idx_o[:], in_=isb[:])

    _split_waits(nc, limit=1)
    return nc


_NC_CACHE = None


def _get_nc():
    global _NC_CACHE
    if _NC_CACHE is None:
        _NC_CACHE = _build()
    return _NC_CACHE


LAST_RES = None


def kernel(z, codebook, _want_timing=False):
    z = np.ascontiguousarray(z, dtype=np.float32)
    codebook = np.ascontiguousarray(codebook, dtype=np.float32)
    assert z.shape == (B, D, H, W) and codebook.shape == (K, D)

    C2 = np.ascontiguousarray(codebook.T) * np.float32(1024.0)   # (512,1024) exact 2^10
    cbh = C2.astype(np.float16)
    cbl = (C2 - cbh.astype(np.float32)).astype(np.float16)
    cn2 = (np.sum(codebook * codebook, axis=1, dtype=np.float32)
           * np.float32(512.0)).reshape(1, K)
    zn2 = (np.einsum("bdhw,bdhw->bhw", z, z, dtype=np.float32).astype(np.float32)
           * np.float32(512.0))

    zh = z.astype(np.float16)
    zl = (z - zh.astype(np.float32)).astype(np.float16)

    zc_view = z.reshape(B, D, H * W)
    zh_view = zh.reshape(B, D, H * W)
    zl_view = zl.reshape(B, D, H * W)
    in_maps = []
    for c in range(N_CORES):
        bsl = slice(c * B_PER_CORE, (c + 1) * B_PER_CORE)
        zn_c = zn2[bsl].reshape(-1)                                    # (8192,)
        znr2 = np.ascontiguousarray(zn_c.reshape(N_SUB, P).T)          # (128, 64)
        in_maps.append(
            dict(
                zh=np.ascontiguousarray(zh_view[bsl]),
                zl=np.ascontiguousarray(zl_view[bsl]),
                zc=np.ascontiguousarray(zc_view[bsl]),
                cbh=cbh,
                cbl=cbl,
                cn2=cn2,
                znr2=znr2,
                cbg=codebook,
            )
        )

    nc = _get_nc()
    res = run_bass_kernel_spmd(nc, in_maps, list(range(N_CORES)), trace=_want_timing)
    global LAST_RES
    LAST_RES = res

    probs = np.concatenate([r["probs"] for r in res.results], axis=0)  # (65536, 1024)
    zq = np.concatenate([r["zq"] for r in res.results], axis=0).reshape(B, D, H, W)

    # q_loss = 1.25 * mean((z_q - z_flat)^2); per-row squared distance equals
    # the fp32 dist at the argmin, which is -mrow * 2^-9.
    tot = 0.0
    for r in res.results:
        tot += -np.sum(r["mrow"].astype(np.float64))
    c_loss = tot / 512.0 / (B * H * W * D)
    q_loss = np.float32(1.25 * c_loss)

    return zq, q_loss, probs


# revision 14
# speedup vs baseline: 1.3824x; 1.3821x over previous
"""VQ codebook kernel for Trainium2, data-parallel over 8 NeuronCores.

Problem (hardcoded shapes): z (16, 512, 64, 64) f32, codebook (1024, 512) f32.
Returns (z_q, q_loss, distance_prob) matching the reference:
    z_flat = z.transpose(0,2,3,1).reshape(-1, 512)          # (65536, 512)
    dist   = ||z||^2 + ||c||^2 - 2 z @ c.T                  # (65536, 1024)
    min_idx = argmin(dist, axis=1)  (first index on fp32 ties)
    distance_prob = softmax(-dist / 0.5, axis=1)
    z_q = z_flat + (codebook[min_idx] - z_flat)   # straight-through, fp32
    q_loss = 1.25 * mean((codebook[min_idx] - z_flat)^2)

Sharding: data-parallel on z rows — core i handles b in {2i, 2i+1} (8192 rows),
codebook replicated.  Scalar loss reduced on host.

Numerics: the argmin must replicate the reference's fp32 rounding of
dist = fl(fl(zn + cn) - 2M) (values ~512, fp32 ulp 6.1e-5) including
first-index tie-breaks, so M needs ~1e-7 accuracy.  The matmul runs as three
fp16 matmuls (Dekker-split z = zh + zl, C2 = 2^10*cb.T = Ch + Cl; the PE
preserves fp16 subnormals) accumulating zh*Ch + zh*Cl + zl*Ch in fp32 PSUM,
giving psum = 2^9 * 2M with ~1e-8 error.  The whole dist pipeline is carried
at 2^9 scale (exact power-of-two) and descaled only in exp (scale=2^-8) and
in the host loss reduction.  Row norms zn may be computed in any fp32 order:
a per-row constant shift moves the whole row rigidly on the fp32 grid within
a binade, preserving the quantized comparison structure.
"""

import sys

for _p in ("/opt/trn_rl_repo", "/root/.axon_site/_ro/trn_rl_repo"):
    if _p not in sys.path:
        sys.path.append(_p)

import numpy as np

import concourse.bass as bass
import concourse.tile as tile
from concourse import mybir
from concourse.bass_utils import run_bass_kernel_spmd
from concourse.masks import make_identity

P = 128
D = 512
K = 1024
B, H, W = 16, 64, 64
N_CORES = 8
B_PER_CORE = B // N_CORES            # 2
ROWS_PER_CORE = B_PER_CORE * H * W   # 8192
N_SUB = ROWS_PER_CORE // P           # 64 subtiles of 128 rows
SUB_PER_BLK = 4                      # 512 rows per z-load block
N_BLK = N_SUB // SUB_PER_BLK         # 16

F32 = mybir.dt.float32
F16 = mybir.dt.float16
U32 = mybir.dt.uint32


def _split_waits(nc, limit=1):
    """This walrus build accepts at most one sync-wait per instruction; move
    extra waits onto preceding same-engine NOPs."""
    for fn in nc.m.functions:
        for blk in fn.blocks:
            new_insts = []
            for inst in blk.instructions:
                si = inst.sync_info
                if si is not None and si.on_wait and len(si.on_wait) > limit:
                    waits = list(si.on_wait)
                    extra, keep = waits[:-limit], waits[-limit:]
                    while extra:
                        chunk, extra = extra[:limit], extra[limit:]
                        nop = mybir.InstNoOp(
                            name=nc.get_next_instruction_name(),
                            engine=inst.engine,
                            ins=[],
                            outs=[],
                            sync_info=mybir.SyncInfo(on_wait=chunk, on_update=[]),
                        )
                        nc.register_instruction(nop)
                        new_insts.append(nop)
                    inst.sync_info = mybir.SyncInfo(
                        on_wait=keep, on_update=list(si.on_update or [])
                    )
                new_insts.append(inst)
            blk.instructions[:] = new_insts


def _build():
    nc = bass.Bass(num_swdge_queues=4)

    zh_in = nc.declare_dram_parameter("zh", [B_PER_CORE, D, H * W], F16, isOutput=False)
    zl_in = nc.declare_dram_parameter("zl", [B_PER_CORE, D, H * W], F16, isOutput=False)
    zc = nc.declare_dram_parameter("zc", [B_PER_CORE, D, H * W], F32, isOutput=False)
    cbh_in = nc.declare_dram_parameter("cbh", [D, K], F16, isOutput=False)
    cbl_in = nc.declare_dram_parameter("cbl", [D, K], F16, isOutput=False)
    cnp = nc.declare_dram_parameter("cn2", [1, K], F32, isOutput=False)  # 2^9*cn
    znr = nc.declare_dram_parameter("znr2", [P, N_SUB], F32, isOutput=False)  # 2^9*zn
    cbg = nc.declare_dram_parameter("cbg", [K, D], F32, isOutput=False)  # gather table

    probs_o = nc.declare_dram_parameter("probs", [ROWS_PER_CORE, K], F32, isOutput=True)
    zq_o = nc.declare_dram_parameter("zq", [B_PER_CORE, D, H * W], F32, isOutput=True)
    m_o = nc.declare_dram_parameter("mrow", [P, N_SUB], F32, isOutput=True)
    idx_o = nc.declare_dram_parameter("idxr", [P, N_SUB], U32, isOutput=True)

    with tile.TileContext(nc) as tc:
        with (
            tc.tile_pool(name="statics", bufs=1) as statics,
            tc.tile_pool(name="zpool", bufs=3) as zpool,
            tc.tile_pool(name="tpool", bufs=3) as tpool,
            tc.tile_pool(name="ndpool", bufs=3) as ndpool,
            tc.tile_pool(name="epool", bufs=3) as epool,
            tc.tile_pool(name="prpool", bufs=3) as prpool,
            tc.tile_pool(name="gqpool", bufs=3) as gqpool,
            tc.tile_pool(name="zqpool", bufs=3) as zqpool,
            tc.tile_pool(name="small", bufs=8) as small,
            tc.tile_pool(name="pspool", bufs=3, space="PSUM") as pspool,
            tc.tile_pool(name="tppool", bufs=2, space="PSUM") as tppool,
        ):
            # resident tensors
            cbh = statics.tile([P, D // P, K], F16)
            nc.sync.dma_start(
                out=cbh[:], in_=cbh_in[:].rearrange("(do di) k -> di do k", di=P)
            )
            cbl = statics.tile([P, D // P, K], F16)
            nc.sync.dma_start(
                out=cbl[:], in_=cbl_in[:].rearrange("(do di) k -> di do k", di=P)
            )
            cn_sb = statics.tile([P, K], F32)
            nc.sync.dma_start(out=cn_sb[:], in_=cnp[:].to_broadcast([P, K]))
            znr_sb = statics.tile([P, N_SUB], F32)
            nc.sync.dma_start(out=znr_sb[:], in_=znr[:])
            ident = statics.tile([P, P], F32)
            make_identity(nc, ident[:])
            msb = statics.tile([P, N_SUB], F32)
            isb = statics.tile([P, N_SUB], U32)

            def emit_tp(gq, zf_s, bb, poff, sub):
                """Transpose gathered z_q to [d, pix].  Emitted two subtiles
                late so the gather chain has finished and the PE never
                stalls."""
                pst = tppool.tile([P, 512], F32)
                for k in range(D // P):
                    nc.tensor.transpose(
                        pst[:, k * P : (k + 1) * P],
                        gq[:, k * P : (k + 1) * P],
                        ident[:],
                    )
                return (pst, zf_s, bb, poff, sub)

            def emit_ste(pst, zf_s, bb, poff, sub):
                """Apply the straight-through fp32 rounding and store.  Emitted
                three subtiles late so the DVE queue never waits on the PE
                transposes."""
                d1 = zqpool.tile([P, D // P, P], F32, tag="d1")
                nc.vector.tensor_tensor(
                    d1[:],
                    pst[:].rearrange("p (do x) -> p do x", do=D // P),
                    zf_s,
                    mybir.AluOpType.subtract,
                )
                zqo = zqpool.tile([P, D // P, P], F32, tag="zqo")
                nc.vector.tensor_tensor(zqo[:], d1[:], zf_s, mybir.AluOpType.add)
                nc.sync.dma_start(
                    out=zq_o[bb].rearrange("(do di) x -> di do x", di=P)[
                        :, :, poff + sub * P : poff + (sub + 1) * P
                    ],
                    in_=zqo[:],
                )

            zspan = SUB_PER_BLK * P

            def load_z_block(blk):
                bb = blk // (N_BLK // B_PER_CORE)
                poff = (blk * SUB_PER_BLK * P) % (H * W)
                zht = zpool.tile([P, D // P, zspan], F16, tag="zh")
                zlt = zpool.tile([P, D // P, zspan], F16, tag="zl")
                zft = zpool.tile([P, D // P, zspan], F32, tag="zf")
                for t_, src in ((zht, zh_in), (zlt, zl_in), (zft, zc)):
                    nc.sync.dma_start(
                        out=t_[:],
                        in_=src[bb].rearrange("(do di) x -> di do x", di=P)[
                            :, :, poff : poff + zspan
                        ],
                    )
                return zht, zlt, zft

            tp_q = []   # gathered z_q awaiting PE transpose (defer 2 subtiles)
            ste_q = []  # transposed z_q awaiting STE + store (defer 3 subtiles)
            zblocks = {0: load_z_block(0)}
            for blk in range(N_BLK):
                bb = blk // (N_BLK // B_PER_CORE)
                poff = (blk * SUB_PER_BLK * P) % (H * W)
                if blk + 1 < N_BLK:
                    zblocks[blk + 1] = load_z_block(blk + 1)
                zht, zlt, zft = zblocks.pop(blk)
                for sub in range(SUB_PER_BLK):
                    st = blk * SUB_PER_BLK + sub
                    pxs = slice(sub * P, (sub + 1) * P)
                    ps = pspool.tile([P, K], F32)
                    for n in range(K // 512):
                        ns = slice(n * 512, (n + 1) * 512)
                        first = True
                        for k in range(D // P):
                            for zt_, cb_ in (
                                (zht, cbh),
                                (zht, cbl),
                                (zlt, cbh),
                            ):
                                nc.tensor.matmul(
                                    ps[:, ns],
                                    lhsT=zt_[:, k, pxs],
                                    rhs=cb_[:, k, ns],
                                    start=first,
                                    stop=(k == D // P - 1 and zt_ is zlt),
                                )
                                first = False
                    if len(tp_q) >= 2:
                        ste_q.append(emit_tp(*tp_q.pop(0)))
                    # t2 = fl(2^9*cn + 2^9*zn) = 2^9 * fl(cn + zn)
                    t = tpool.tile([P, K], F32)
                    nc.scalar.activation(
                        t[:], cn_sb[:], mybir.ActivationFunctionType.Identity,
                        bias=znr_sb[:, st : st + 1], scale=1.0,
                    )
                    # negdist2 = fl(psum - t2) = 2^9 * (-dist)
                    nd = ndpool.tile([P, K], F32)
                    nc.vector.tensor_tensor(
                        nd[:], ps[:], t[:], mybir.AluOpType.subtract
                    )
                    # first-index argmax of negdist == argmin of dist
                    m8 = small.tile([P, 8], F32)
                    nc.vector.max(m8[:], nd[:])
                    idx8 = small.tile([P, 8], U32)
                    nc.vector.max_index(idx8[:], m8[:], nd[:])
                    # e = exp(2^-8*negdist2 - 2^-8*m2) = exp(-2*(dist-rowmin))
                    biasm = small.tile([P, 1], F32)
                    nc.scalar.mul(biasm[:], m8[:, :1], -0.00390625)  # -2^-8
                    e = epool.tile([P, K], F32)
                    s = small.tile([P, 1], F32)
                    nc.scalar.activation(
                        e[:], nd[:], mybir.ActivationFunctionType.Exp,
                        bias=biasm[:], scale=0.00390625, accum_out=s[:],
                    )
                    r = small.tile([P, 1], F32)
                    nc.vector.reciprocal(r[:], s[:])
                    pr = prpool.tile([P, K], F32)
                    nc.scalar.mul(pr[:], e[:], r[:])
                    nc.sync.dma_start(
                        out=probs_o[st * P : (st + 1) * P, :], in_=pr[:]
                    )
                    # z_q gather; transpose/STE/store deferred one subtile
                    gq = gqpool.tile([P, D], F32)
                    nc.gpsimd.indirect_dma_start(
                        out=gq[:], out_offset=None,
                        in_=cbg[:],
                        in_offset=bass.IndirectOffsetOnAxis(ap=idx8[:, :1], axis=0),
                    )
                    tp_q.append((gq, zft[:, :, pxs], bb, poff, sub))
                    # stash per-row max(negdist2) and idx
                    nc.scalar.copy(msb[:, st : st + 1], m8[:, :1])
                    nc.gpsimd.tensor_copy(isb[:, st : st + 1], idx8[:, :1])
                    if len(ste_q) >= 2:
                        emit_ste(*ste_q.pop(0))

            while tp_q:
                ste_q.append(emit_tp(*tp_q.pop(0)))
            while ste_q:
                emit_ste(*ste_q.pop(0))
            nc.sync.dma_start(out=m_o[:], in_=msb[:])
            nc.sync.dma_start(out=idx_o[:], in_=isb[:])

    _split_waits(nc, limit=1)
    return nc


_NC_CACHE = None


def _get_nc():
    global _NC_CACHE
    if _NC_CACHE is None:
        _NC_CACHE = _build()
    return _NC_CACHE


LAST_RES = None


def kernel(z, codebook, _want_timing=False):
    z = np.ascontiguousarray(z, dtype=np.float32)
    codebook = np.ascontiguousarray(codebook, dtype=np.float32)
    assert z.shape == (B, D, H, W) and codebook.shape == (K, D)

    C2 = np.ascontiguousarray(codebook.T) * np.float32(1024.0)   # (512,1024) exact 2^10
    cbh = C2.astype(np.float16)
    cbl = (C2 - cbh.astype(np.float32)).astype(np.float16)
    cn2 = (np.sum(codebook * codebook, axis=1, dtype=np.float32)
           * np.float32(512.0)).reshape(1, K)
    zn2 = (np.einsum("bdhw,bdhw->bhw", z, z, dtype=np.float32).astype(np.float32)
           * np.float32(512.0))

    zh = z.astype(np.float16)
    zl = (z - zh.astype(np.float32)).astype(np.float16)

    zc_view = z.reshape(B, D, H * W)
    zh_view = zh.reshape(B, D, H * W)
    zl_view = zl.reshape(B, D, H * W)
    in_maps = []
    for c in range(N_CORES):
        bsl = slice(c * B_PER_CORE, (c + 1) * B_PER_CORE)
        zn_c = zn2[bsl].reshape(-1)                                    # (8192,)
        znr2 = np.ascontiguousarray(zn_c.reshape(N_SUB, P).T)          # (128, 64)
        in_maps.append(
            dict(
                zh=np.ascontiguousarray(zh_view[bsl]),
                zl=np.ascontiguousarray(zl_view[bsl]),
                zc=np.ascontiguousarray(zc_view[bsl]),
                cbh=cbh,
                cbl=cbl,
                cn2=cn2,
                znr2=znr2,
                cbg=codebook,
            )
        )

    nc = _get_nc()
    res = run_bass_kernel_spmd(nc, in_maps, list(range(N_CORES)), trace=_want_timing)
    global LAST_RES
    LAST_RES = res

    probs = np.concatenate([r["probs"] for r in res.results], axis=0)  # (65536, 1024)
    zq = np.concatenate([r["zq"] for r in res.results], axis=0).reshape(B, D, H, W)

    # q_loss = 1.25 * mean((z_q - z_flat)^2); per-row squared distance equals
    # the fp32 dist at the argmin, which is -mrow * 2^-9.
    tot = 0.0
    for r in res.results:
        tot += -np.sum(r["mrow"].astype(np.float64))
    c_loss = tot / 512.0 / (B * H * W * D)
    q_loss = np.float32(1.25 * c_loss)

    return zq, q_loss, probs


# revision 21
# speedup vs baseline: 1.4064x; 1.0174x over previous
"""VQ codebook kernel for Trainium2, data-parallel over 8 NeuronCores.

Problem (hardcoded shapes): z (16, 512, 64, 64) f32, codebook (1024, 512) f32.
Returns (z_q, q_loss, distance_prob) matching the reference:
    z_flat = z.transpose(0,2,3,1).reshape(-1, 512)          # (65536, 512)
    dist   = ||z||^2 + ||c||^2 - 2 z @ c.T                  # (65536, 1024)
    min_idx = argmin(dist, axis=1)  (first index on fp32 ties)
    distance_prob = softmax(-dist / 0.5, axis=1)
    z_q = z_flat + (codebook[min_idx] - z_flat)   # straight-through, fp32
    q_loss = 1.25 * mean((codebook[min_idx] - z_flat)^2)

Sharding: data-parallel on z rows — core i handles b in {2i, 2i+1} (8192 rows),
codebook replicated.  Scalar loss reduced on host.

Numerics: the argmin must replicate the reference's fp32 rounding of
dist = fl(fl(zn + cn) - 2M) (values ~512, fp32 ulp 6.1e-5) including
first-index tie-breaks, so M needs ~1e-7 accuracy.  The matmul runs as three
fp16 matmuls (Dekker-split z = zh + zl, C2 = 2^10*cb.T = Ch + Cl; the PE
preserves fp16 subnormals) accumulating zh*Ch + zh*Cl + zl*Ch in fp32 PSUM,
giving psum = 2^9 * 2M with ~1e-8 error.  The whole dist pipeline is carried
at 2^9 scale (exact power-of-two) and descaled only in exp (scale=2^-8) and
in the host loss reduction.  Row norms zn may be computed in any fp32 order:
a per-row constant shift moves the whole row rigidly on the fp32 grid within
a binade, preserving the quantized comparison structure.
"""

import sys

for _p in ("/opt/trn_rl_repo", "/root/.axon_site/_ro/trn_rl_repo"):
    if _p not in sys.path:
        sys.path.append(_p)

import numpy as np

import concourse.bass as bass
import concourse.tile as tile
from concourse import mybir
from concourse.bass_utils import run_bass_kernel_spmd
from concourse.masks import make_identity

P = 128
D = 512
K = 1024
B, H, W = 16, 64, 64
N_CORES = 8
B_PER_CORE = B // N_CORES            # 2
ROWS_PER_CORE = B_PER_CORE * H * W   # 8192
N_SUB = ROWS_PER_CORE // P           # 64 subtiles of 128 rows
SUB_PER_BLK = 4                      # 512 rows per z-load block
N_BLK = N_SUB // SUB_PER_BLK         # 16

F32 = mybir.dt.float32
F16 = mybir.dt.float16
U32 = mybir.dt.uint32


def _split_waits(nc, limit=1):
    """This walrus build accepts at most one sync-wait per instruction; move
    extra waits onto preceding same-engine NOPs."""
    for fn in nc.m.functions:
        for blk in fn.blocks:
            new_insts = []
            for inst in blk.instructions:
                si = inst.sync_info
                if si is not None and si.on_wait and len(si.on_wait) > limit:
                    waits = list(si.on_wait)
                    extra, keep = waits[:-limit], waits[-limit:]
                    while extra:
                        chunk, extra = extra[:limit], extra[limit:]
                        nop = mybir.InstNoOp(
                            name=nc.get_next_instruction_name(),
                            engine=inst.engine,
                            ins=[],
                            outs=[],
                            sync_info=mybir.SyncInfo(on_wait=chunk, on_update=[]),
                        )
                        nc.register_instruction(nop)
                        new_insts.append(nop)
                    inst.sync_info = mybir.SyncInfo(
                        on_wait=keep, on_update=list(si.on_update or [])
                    )
                new_insts.append(inst)
            blk.instructions[:] = new_insts


def _build():
    nc = bass.Bass(num_swdge_queues=4)

    zh_in = nc.declare_dram_parameter("zh", [B_PER_CORE, D, H * W], F16, isOutput=False)
    zl_in = nc.declare_dram_parameter("zl", [B_PER_CORE, D, H * W], F16, isOutput=False)
    zc = nc.declare_dram_parameter("zc", [B_PER_CORE, D, H * W], F32, isOutput=False)
    cbh_in = nc.declare_dram_parameter("cbh", [D, K], F16, isOutput=False)
    cbl_in = nc.declare_dram_parameter("cbl", [D, K], F16, isOutput=False)
    cnp = nc.declare_dram_parameter("cn2", [1, K], F32, isOutput=False)  # 2^9*cn
    znr = nc.declare_dram_parameter("znr2", [P, N_SUB], F32, isOutput=False)  # 2^9*zn
    cbg = nc.declare_dram_parameter("cbg", [K, D], F32, isOutput=False)  # gather table

    probs_o = nc.declare_dram_parameter("probs", [ROWS_PER_CORE, K], F32, isOutput=True)
    zq_o = nc.declare_dram_parameter("zq", [B_PER_CORE, D, H * W], F32, isOutput=True)
    m_o = nc.declare_dram_parameter("mrow", [P, N_SUB], F32, isOutput=True)
    idx_o = nc.declare_dram_parameter("idxr", [P, N_SUB], U32, isOutput=True)

    with tile.TileContext(nc) as tc:
        with (
            tc.tile_pool(name="statics", bufs=1) as statics,
            tc.tile_pool(name="zpool", bufs=3) as zpool,
            tc.tile_pool(name="tpool", bufs=3) as tpool,
            tc.tile_pool(name="ndpool", bufs=3) as ndpool,
            tc.tile_pool(name="epool", bufs=3) as epool,
            tc.tile_pool(name="prpool", bufs=3) as prpool,
            tc.tile_pool(name="gqpool", bufs=4) as gqpool,
            tc.tile_pool(name="zqpool", bufs=3) as zqpool,
            tc.tile_pool(name="small", bufs=8) as small,
            tc.tile_pool(name="pspool", bufs=3, space="PSUM") as pspool,
            tc.tile_pool(name="tppool", bufs=2, space="PSUM") as tppool,
        ):
            # resident tensors
            cbh = statics.tile([P, D // P, K], F16)
            nc.sync.dma_start(
                out=cbh[:], in_=cbh_in[:].rearrange("(do di) k -> di do k", di=P)
            )
            cbl = statics.tile([P, D // P, K], F16)
            nc.sync.dma_start(
                out=cbl[:], in_=cbl_in[:].rearrange("(do di) k -> di do k", di=P)
            )
            cn_sb = statics.tile([P, K], F32)
            nc.sync.dma_start(out=cn_sb[:], in_=cnp[:].to_broadcast([P, K]))
            znr_sb = statics.tile([P, N_SUB], F32)
            nc.sync.dma_start(out=znr_sb[:], in_=znr[:])
            ident = statics.tile([P, P], F32)
            make_identity(nc, ident[:])
            msb = statics.tile([P, N_SUB], F32)
            isb = statics.tile([P, N_SUB], U32)

            def emit_tp(gq, zf_s, bb, poff, sub):
                """Transpose gathered z_q to [d, pix].  Emitted two subtiles
                late so the gather chain has finished and the PE never
                stalls."""
                pst = tppool.tile([P, 512], F32)
                for k in range(D // P):
                    nc.tensor.transpose(
                        pst[:, k * P : (k + 1) * P],
                        gq[:, k * P : (k + 1) * P],
                        ident[:],
                    )
                return (pst, zf_s, bb, poff, sub)

            def emit_ste(pst, zf_s, bb, poff, sub):
                """Apply the straight-through fp32 rounding and store.  Emitted
                three subtiles late so the DVE queue never waits on the PE
                transposes."""
                d1 = zqpool.tile([P, D // P, P], F32, tag="d1")
                nc.vector.tensor_tensor(
                    d1[:],
                    pst[:].rearrange("p (do x) -> p do x", do=D // P),
                    zf_s,
                    mybir.AluOpType.subtract,
                )
                zqo = zqpool.tile([P, D // P, P], F32, tag="zqo")
                nc.vector.tensor_tensor(zqo[:], d1[:], zf_s, mybir.AluOpType.add)
                nc.gpsimd.dma_start(
                    out=zq_o[bb].rearrange("(do di) x -> di do x", di=P)[
                        :, :, poff + sub * P : poff + (sub + 1) * P
                    ],
                    in_=zqo[:],
                )

            zspan = SUB_PER_BLK * P

            def load_z_block(blk):
                bb = blk // (N_BLK // B_PER_CORE)
                poff = (blk * SUB_PER_BLK * P) % (H * W)
                zht = zpool.tile([P, D // P, zspan], F16, tag="zh")
                zlt = zpool.tile([P, D // P, zspan], F16, tag="zl")
                zft = zpool.tile([P, D // P, zspan], F32, tag="zf")
                for t_, src in ((zht, zh_in), (zlt, zl_in), (zft, zc)):
                    nc.sync.dma_start(
                        out=t_[:],
                        in_=src[bb].rearrange("(do di) x -> di do x", di=P)[
                            :, :, poff : poff + zspan
                        ],
                    )
                return zht, zlt, zft

            tp_q = []   # gathered z_q awaiting PE transpose (defer 2 subtiles)
            ste_q = []  # transposed z_q awaiting STE + store (defer 3 subtiles)
            zblocks = {0: load_z_block(0)}
            for blk in range(N_BLK):
                bb = blk // (N_BLK // B_PER_CORE)
                poff = (blk * SUB_PER_BLK * P) % (H * W)
                if blk + 1 < N_BLK:
                    zblocks[blk + 1] = load_z_block(blk + 1)
                zht, zlt, zft = zblocks.pop(blk)
                for sub in range(SUB_PER_BLK):
                    st = blk * SUB_PER_BLK + sub
                    pxs = slice(sub * P, (sub + 1) * P)
                    ps = pspool.tile([P, K], F32)
                    for n in range(K // 512):
                        ns = slice(n * 512, (n + 1) * 512)
                        first = True
                        for k in range(D // P):
                            for zt_, cb_ in (
                                (zht, cbh),
                                (zht, cbl),
                                (zlt, cbh),
                            ):
                                nc.tensor.matmul(
                                    ps[:, ns],
                                    lhsT=zt_[:, k, pxs],
                                    rhs=cb_[:, k, ns],
                                    start=first,
                                    stop=(k == D // P - 1 and zt_ is zlt),
                                )
                                first = False
                    if len(tp_q) >= 2:
                        ste_q.append(emit_tp(*tp_q.pop(0)))
                    # t2 = fl(2^9*cn + 2^9*zn) = 2^9 * fl(cn + zn)
                    t = tpool.tile([P, K], F32)
                    nc.scalar.activation(
                        t[:], cn_sb[:], mybir.ActivationFunctionType.Identity,
                        bias=znr_sb[:, st : st + 1], scale=1.0,
                    )
                    # negdist2 = fl(psum - t2) = 2^9 * (-dist)
                    nd = ndpool.tile([P, K], F32)
                    nc.vector.tensor_tensor(
                        nd[:], ps[:], t[:], mybir.AluOpType.subtract
                    )
                    # first-index argmax of negdist == argmin of dist
                    m8 = small.tile([P, 8], F32)
                    nc.vector.max(m8[:], nd[:])
                    idx8 = small.tile([P, 8], U32)
                    nc.vector.max_index(idx8[:], m8[:], nd[:])
                    # e = exp(2^-8*negdist2 - 2^-8*m2) = exp(-2*(dist-rowmin))
                    # The bias doubles as the per-row loss stash: msb holds
                    # -2^-8 * max(negdist2); host recovers rowmin as 2^8*msb.
                    nc.scalar.mul(msb[:, st : st + 1], m8[:, :1], -0.00390625)
                    e = epool.tile([P, K], F32)
                    s = small.tile([P, 1], F32)
                    nc.scalar.activation(
                        e[:], nd[:], mybir.ActivationFunctionType.Exp,
                        bias=msb[:, st : st + 1], scale=0.00390625, accum_out=s[:],
                    )
                    r = small.tile([P, 1], F32)
                    nc.vector.reciprocal(r[:], s[:])
                    pr = prpool.tile([P, K], F32)
                    nc.scalar.mul(pr[:], e[:], r[:])
                    nc.scalar.dma_start(
                        out=probs_o[st * P : (st + 1) * P, :], in_=pr[:]
                    )
                    # z_q gather; transpose/STE/store deferred one subtile
                    gq = gqpool.tile([P, D], F32)
                    nc.gpsimd.indirect_dma_start(
                        out=gq[:], out_offset=None,
                        in_=cbg[:],
                        in_offset=bass.IndirectOffsetOnAxis(ap=idx8[:, :1], axis=0),
                    )
                    tp_q.append((gq, zft[:, :, pxs], bb, poff, sub))
                    nc.gpsimd.tensor_copy(isb[:, st : st + 1], idx8[:, :1])
                    if len(ste_q) >= 2:
                        emit_ste(*ste_q.pop(0))

            while tp_q:
                ste_q.append(emit_tp(*tp_q.pop(0)))
            while ste_q:
                emit_ste(*ste_q.pop(0))
            nc.sync.dma_start(out=m_o[:], in_=msb[:])
            nc.sync.dma_start(out=idx_o[:], in_=isb[:])

    _split_waits(nc, limit=1)
    return nc


_NC_CACHE = None


def _get_nc():
    global _NC_CACHE
    if _NC_CACHE is None:
        _NC_CACHE = _build()
    return _NC_CACHE


LAST_RES = None


def kernel(z, codebook, _want_timing=False):
    z = np.ascontiguousarray(z, dtype=np.float32)
    codebook = np.ascontiguousarray(codebook, dtype=np.float32)
    assert z.shape == (B, D, H, W) and codebook.shape == (K, D)

    C2 = np.ascontiguousarray(codebook.T) * np.float32(1024.0)   # (512,1024) exact 2^10
    cbh = C2.astype(np.float16)
    cbl = (C2 - cbh.astype(np.float32)).astype(np.float16)
    cn2 = (np.sum(codebook * codebook, axis=1, dtype=np.float32)
           * np.float32(512.0)).reshape(1, K)
    zn2 = (np.einsum("bdhw,bdhw->bhw", z, z, dtype=np.float32).astype(np.float32)
           * np.float32(512.0))

    zh = z.astype(np.float16)
    zl = (z - zh.astype(np.float32)).astype(np.float16)

    zc_view = z.reshape(B, D, H * W)
    zh_view = zh.reshape(B, D, H * W)
    zl_view = zl.reshape(B, D, H * W)
    in_maps = []
    for c in range(N_CORES):
        bsl = slice(c * B_PER_CORE, (c + 1) * B_PER_CORE)
        zn_c = zn2[bsl].reshape(-1)                                    # (8192,)
        znr2 = np.ascontiguousarray(zn_c.reshape(N_SUB, P).T)          # (128, 64)
        in_maps.append(
            dict(
                zh=np.ascontiguousarray(zh_view[bsl]),
                zl=np.ascontiguousarray(zl_view[bsl]),
                zc=np.ascontiguousarray(zc_view[bsl]),
                cbh=cbh,
                cbl=cbl,
                cn2=cn2,
                znr2=znr2,
                cbg=codebook,
            )
        )

    nc = _get_nc()
    res = run_bass_kernel_spmd(nc, in_maps, list(range(N_CORES)), trace=_want_timing)
    global LAST_RES
    LAST_RES = res

    probs = np.concatenate([r["probs"] for r in res.results], axis=0)  # (65536, 1024)
    zq = np.concatenate([r["zq"] for r in res.results], axis=0).reshape(B, D, H, W)

    # q_loss = 1.25 * mean((z_q - z_flat)^2); per-row squared distance equals
    # the fp32 dist at the argmin; mrow holds -2^-8 * (2^9 * -dist_min), so
    # dist_min = mrow / 2.
    tot = 0.0
    for r in res.results:
        tot += np.sum(r["mrow"].astype(np.float64))
    c_loss = tot / 2.0 / (B * H * W * D)
    q_loss = np.float32(1.25 * c_loss)

    return zq, q_loss, probs


# revision 27
# speedup vs baseline: 1.5622x; 1.1108x over previous
"""VQ codebook kernel for Trainium2, data-parallel over 8 NeuronCores.

Problem (hardcoded shapes): z (16, 512, 64, 64) f32, codebook (1024, 512) f32.
Returns (z_q, q_loss, distance_prob) matching the reference:
    z_flat = z.transpose(0,2,3,1).reshape(-1, 512)          # (65536, 512)
    dist   = ||z||^2 + ||c||^2 - 2 z @ c.T                  # (65536, 1024)
    min_idx = argmin(dist, axis=1)  (first index on fp32 ties)
    distance_prob = softmax(-dist / 0.5, axis=1)
    z_q = z_flat + (codebook[min_idx] - z_flat)   # straight-through, fp32
    q_loss = 1.25 * mean((codebook[min_idx] - z_flat)^2)

Sharding: data-parallel on z rows — core i handles b in {2i, 2i+1} (8192 rows),
codebook replicated.  Scalar loss reduced on host.

Numerics: the reference's dist values (~512, fp32 ulp 6.1e-5) must be
reproduced through the exact rounding sequence fl(fl(zn+cn) - 2M), including
first-index argmin tie-breaks.  The device runs the matmul in float32r
(full PE rate, |dist error| <~ 2e-5 — a third of an ulp), which yields the
correct quantized argmin for every row whose top-2 gap is not tiny.  The
device emits the top-8 negdist values per row (max8); rows whose top-2 gap
is below a safety margin (8 ulps, ~2% of rows — vastly larger than the f32r
error) are re-checked on the host with an exact fp32 computation, and the
few that flipped get their z_q/idx patched host-side.  distance_prob and
q_loss tolerate the f32r error (~3e-5 relative) directly.  Row norms zn may
be computed in any fp32 order: a per-row constant shift moves the whole row
rigidly on the fp32 grid within a binade, preserving the comparison
structure.  z_q replicates the straight-through double rounding
fl(z + fl(zq - z)) elementwise on device.
"""

import sys

for _p in ("/opt/trn_rl_repo", "/root/.axon_site/_ro/trn_rl_repo"):
    if _p not in sys.path:
        sys.path.append(_p)

import numpy as np

import concourse.bass as bass
import concourse.tile as tile
from concourse import mybir
from concourse.bass_utils import run_bass_kernel_spmd
from concourse.masks import make_identity

P = 128
D = 512
K = 1024
B, H, W = 16, 64, 64
N_CORES = 8
B_PER_CORE = B // N_CORES            # 2
ROWS_PER_CORE = B_PER_CORE * H * W   # 8192
N_SUB = ROWS_PER_CORE // P           # 64 subtiles of 128 rows
SUB_PER_BLK = 4                      # 512 rows per z-load block
N_BLK = N_SUB // SUB_PER_BLK         # 16

F32 = mybir.dt.float32
F32R = mybir.dt.float32r
U32 = mybir.dt.uint32

ULP512 = 6.103515625e-05             # fp32 ulp in [512, 1024)
MARGIN = 8 * ULP512                  # host re-check when top-2 gap is below


def _split_waits(nc, limit=1):
    """This walrus build accepts at most one sync-wait per instruction; move
    extra waits onto preceding same-engine NOPs."""
    for fn in nc.m.functions:
        for blk in fn.blocks:
            new_insts = []
            for inst in blk.instructions:
                si = inst.sync_info
                if si is not None and si.on_wait and len(si.on_wait) > limit:
                    waits = list(si.on_wait)
                    extra, keep = waits[:-limit], waits[-limit:]
                    while extra:
                        chunk, extra = extra[:limit], extra[limit:]
                        nop = mybir.InstNoOp(
                            name=nc.get_next_instruction_name(),
                            engine=inst.engine,
                            ins=[],
                            outs=[],
                            sync_info=mybir.SyncInfo(on_wait=chunk, on_update=[]),
                        )
                        nc.register_instruction(nop)
                        new_insts.append(nop)
                    inst.sync_info = mybir.SyncInfo(
                        on_wait=keep, on_update=list(si.on_update or [])
                    )
                new_insts.append(inst)
            blk.instructions[:] = new_insts


def _build():
    nc = bass.Bass(num_swdge_queues=4)

    zc = nc.declare_dram_parameter("zc", [B_PER_CORE, D, H * W], F32R, isOutput=False)
    cbt2 = nc.declare_dram_parameter("cbt2", [D, K], F32R, isOutput=False)  # 2*cb.T
    cnp = nc.declare_dram_parameter("cn", [1, K], F32, isOutput=False)
    znr = nc.declare_dram_parameter("znr", [P, N_SUB], F32, isOutput=False)
    cbg = nc.declare_dram_parameter("cbg", [K, D], F32, isOutput=False)  # gather table

    probs_o = nc.declare_dram_parameter("probs", [ROWS_PER_CORE, K], F32, isOutput=True)
    zq_o = nc.declare_dram_parameter("zq", [B_PER_CORE, D, H * W], F32, isOutput=True)
    m_o = nc.declare_dram_parameter("mrow", [P, N_SUB], F32, isOutput=True)
    g_o = nc.declare_dram_parameter("grow", [P, N_SUB], F32, isOutput=True)  # 2nd best
    idx_o = nc.declare_dram_parameter("idxr", [P, N_SUB], U32, isOutput=True)

    with tile.TileContext(nc) as tc:
        with (
            tc.tile_pool(name="statics", bufs=1) as statics,
            tc.tile_pool(name="zpool", bufs=3) as zpool,
            tc.tile_pool(name="tpool", bufs=3) as tpool,
            tc.tile_pool(name="ndpool", bufs=3) as ndpool,
            tc.tile_pool(name="epool", bufs=3) as epool,
            tc.tile_pool(name="prpool", bufs=3) as prpool,
            tc.tile_pool(name="gqpool", bufs=4) as gqpool,
            tc.tile_pool(name="zqpool", bufs=3) as zqpool,
            tc.tile_pool(name="small", bufs=8) as small,
            tc.tile_pool(name="pspool", bufs=3, space="PSUM") as pspool,
            tc.tile_pool(name="tppool", bufs=2, space="PSUM") as tppool,
        ):
            # resident tensors
            cbt = statics.tile([P, D // P, K], F32R)       # 2*cb.T as [di, do, k]
            nc.sync.dma_start(
                out=cbt[:],
                in_=cbt2[:].rearrange("(do di) k -> di do k", di=P),
            )
            cn_sb = statics.tile([P, K], F32)
            nc.sync.dma_start(out=cn_sb[:], in_=cnp[:].to_broadcast([P, K]))
            znr_sb = statics.tile([P, N_SUB], F32)
            nc.sync.dma_start(out=znr_sb[:], in_=znr[:])
            ident = statics.tile([P, P], F32)
            make_identity(nc, ident[:])
            msb = statics.tile([P, N_SUB], F32)
            gsb = statics.tile([P, N_SUB], F32)
            isb = statics.tile([P, N_SUB], U32)

            zspan = SUB_PER_BLK * P

            def load_z_block(blk):
                bb = blk // (N_BLK // B_PER_CORE)
                poff = (blk * SUB_PER_BLK * P) % (H * W)
                zft = zpool.tile([P, D // P, zspan], F32R, tag="zf")
                nc.sync.dma_start(
                    out=zft[:],
                    in_=zc[bb].rearrange("(do di) x -> di do x", di=P)[
                        :, :, poff : poff + zspan
                    ],
                )
                return zft

            def emit_tp(gq, zf_s, bb, poff, sub):
                """Transpose gathered z_q to [d, pix].  Emitted two subtiles
                late so the gather chain has finished and the PE never
                stalls."""
                pst = tppool.tile([P, 512], F32)
                for k in range(D // P):
                    nc.tensor.transpose(
                        pst[:, k * P : (k + 1) * P],
                        gq[:, k * P : (k + 1) * P],
                        ident[:],
                    )
                return (pst, zf_s, bb, poff, sub)

            def emit_ste(pst, zf_s, bb, poff, sub):
                """Apply the straight-through fp32 rounding and store.  Emitted
                three subtiles late so the DVE queue never waits on the PE
                transposes."""
                d1 = zqpool.tile([P, D // P, P], F32, tag="d1")
                nc.vector.tensor_tensor(
                    d1[:],
                    pst[:].rearrange("p (do x) -> p do x", do=D // P),
                    zf_s,
                    mybir.AluOpType.subtract,
                )
                zqo = zqpool.tile([P, D // P, P], F32, tag="zqo")
                nc.vector.tensor_tensor(zqo[:], d1[:], zf_s, mybir.AluOpType.add)
                nc.gpsimd.dma_start(
                    out=zq_o[bb].rearrange("(do di) x -> di do x", di=P)[
                        :, :, poff + sub * P : poff + (sub + 1) * P
                    ],
                    in_=zqo[:],
                )

            tp_q = []   # gathered z_q awaiting PE transpose (defer 2 subtiles)
            ste_q = []  # transposed z_q awaiting STE + store (defer 3 subtiles)
            zblocks = {0: load_z_block(0)}
            for blk in range(N_BLK):
                bb = blk // (N_BLK // B_PER_CORE)
                poff = (blk * SUB_PER_BLK * P) % (H * W)
                if blk + 1 < N_BLK:
                    zblocks[blk + 1] = load_z_block(blk + 1)
                zft = zblocks.pop(blk)
                for sub in range(SUB_PER_BLK):
                    st = blk * SUB_PER_BLK + sub
                    pxs = slice(sub * P, (sub + 1) * P)
                    ps = pspool.tile([P, K], F32)
                    for n in range(K // 512):
                        ns = slice(n * 512, (n + 1) * 512)
                        for k in range(D // P):
                            nc.tensor.matmul(
                                ps[:, ns],
                                lhsT=zft[:, k, pxs],
                                rhs=cbt[:, k, ns],
                                start=(k == 0),
                                stop=(k == D // P - 1),
                            )
                    if len(tp_q) >= 2:
                        ste_q.append(emit_tp(*tp_q.pop(0)))
                    # t = fl(cn + zn)  (bit-exact fp32 add on ACT)
                    t = tpool.tile([P, K], F32)
                    nc.scalar.activation(
                        t[:], cn_sb[:], mybir.ActivationFunctionType.Identity,
                        bias=znr_sb[:, st : st + 1], scale=1.0,
                    )
                    # negdist = fl(2M - t) = -dist  (f32r-approximate M)
                    nd = ndpool.tile([P, K], F32)
                    nc.vector.tensor_tensor(
                        nd[:], ps[:], t[:], mybir.AluOpType.subtract
                    )
                    # first-index argmax of negdist == argmin of dist
                    m8 = small.tile([P, 8], F32)
                    nc.vector.max(m8[:], nd[:])
                    idx8 = small.tile([P, 8], U32)
                    nc.vector.max_index(idx8[:], m8[:], nd[:])
                    # e = exp(2*negdist - 2*m) = exp(-2*(dist-rowmin))
                    # The bias doubles as the per-row loss stash: msb holds
                    # -2 * max(negdist); host recovers rowmin as msb/2.
                    nc.scalar.mul(msb[:, st : st + 1], m8[:, :1], -2.0)
                    nc.gpsimd.tensor_copy(gsb[:, st : st + 1], m8[:, 1:2])
                    e = epool.tile([P, K], F32)
                    s = small.tile([P, 1], F32)
                    nc.scalar.activation(
                        e[:], nd[:], mybir.ActivationFunctionType.Exp,
                        bias=msb[:, st : st + 1], scale=2.0, accum_out=s[:],
                    )
                    r = small.tile([P, 1], F32)
                    nc.vector.reciprocal(r[:], s[:])
                    pr = prpool.tile([P, K], F32)
                    nc.scalar.mul(pr[:], e[:], r[:])
                    nc.scalar.dma_start(
                        out=probs_o[st * P : (st + 1) * P, :], in_=pr[:]
                    )
                    # z_q gather; transpose/STE/store deferred (see emit_*)
                    gq = gqpool.tile([P, D], F32)
                    nc.gpsimd.indirect_dma_start(
                        out=gq[:], out_offset=None,
                        in_=cbg[:],
                        in_offset=bass.IndirectOffsetOnAxis(ap=idx8[:, :1], axis=0),
                    )
                    tp_q.append((gq, zft[:, :, pxs].bitcast(F32), bb, poff, sub))
                    nc.gpsimd.tensor_copy(isb[:, st : st + 1], idx8[:, :1])
                    if len(ste_q) >= 2:
                        emit_ste(*ste_q.pop(0))

            while tp_q:
                ste_q.append(emit_tp(*tp_q.pop(0)))
            while ste_q:
                emit_ste(*ste_q.pop(0))
            nc.sync.dma_start(out=m_o[:], in_=msb[:])
            nc.sync.dma_start(out=g_o[:], in_=gsb[:])
            nc.sync.dma_start(out=idx_o[:], in_=isb[:])

    _split_waits(nc, limit=1)
    return nc


_NC_CACHE = None


def _get_nc():
    global _NC_CACHE
    if _NC_CACHE is None:
        _NC_CACHE = _build()
    return _NC_CACHE


LAST_RES = None
LAST_NFRAGILE = None
LAST_NFLIPS = None


def kernel(z, codebook, _want_timing=False):
    z = np.ascontiguousarray(z, dtype=np.float32)
    codebook = np.ascontiguousarray(codebook, dtype=np.float32)
    assert z.shape == (B, D, H, W) and codebook.shape == (K, D)

    cbt2 = np.ascontiguousarray((2.0 * codebook).T)            # (512, 1024)
    cn = np.sum(codebook * codebook, axis=1, dtype=np.float32).reshape(1, K)
    zn = np.einsum("bdhw,bdhw->bhw", z, z, dtype=np.float32).astype(np.float32)

    zc_view = z.reshape(B, D, H * W)
    in_maps = []
    for c in range(N_CORES):
        bsl = slice(c * B_PER_CORE, (c + 1) * B_PER_CORE)
        zn_c = zn[bsl].reshape(-1)                                     # (8192,)
        znr = np.ascontiguousarray(zn_c.reshape(N_SUB, P).T)           # (128, 64)
        in_maps.append(
            dict(
                zc=np.ascontiguousarray(zc_view[bsl]),
                cbt2=cbt2,
                cn=cn,
                znr=znr,
                cbg=codebook,
            )
        )

    nc = _get_nc()
    res = run_bass_kernel_spmd(nc, in_maps, list(range(N_CORES)), trace=_want_timing)
    global LAST_RES
    LAST_RES = res

    probs = np.concatenate([r["probs"] for r in res.results], axis=0)  # (65536, 1024)
    zq = np.concatenate([r["zq"] for r in res.results], axis=0).reshape(B, D, H * W)

    # per-row device results, in row order (row = subtile*128 + partition)
    def rows(a):  # [P, N_SUB] -> (ROWS_PER_CORE,)
        return np.ascontiguousarray(a.T).reshape(-1)

    neg_m = np.concatenate([rows(r["mrow"]) for r in res.results])   # -2*max -> m = -x/2
    m1 = -0.5 * neg_m.astype(np.float64)                             # best negdist
    m2 = np.concatenate([rows(r["grow"]) for r in res.results]).astype(np.float64)
    idx = np.concatenate([rows(r["idxr"]) for r in res.results]).astype(np.int64)

    # ---- host re-check of fragile rows (tiny top-2 gap in the f32r dist) ----
    fragile = np.where((m1 - m2) < MARGIN)[0]
    global LAST_NFRAGILE, LAST_NFLIPS
    LAST_NFRAGILE = len(fragile)
    LAST_NFLIPS = 0
    if len(fragile):
        bi, hwi = np.divmod(fragile, H * W)
        z_rows = np.ascontiguousarray(
            z.reshape(B, D, H * W)[bi, :, hwi], dtype=np.float32
        )  # (F, 512) exact z_flat rows
        Mf = z_rows @ codebook.T                                   # fp32 sgemm
        zn_f = zn.reshape(-1)[fragile][:, None]
        dist_f = (zn_f + cn) - 2.0 * Mf                            # fp32, ref rounding
        idx_exact = dist_f.argmin(axis=1)
        flips = np.where(idx_exact != idx[fragile])[0]
        LAST_NFLIPS = len(flips)
        if len(flips):
            rowsel = fragile[flips]
            newidx = idx_exact[flips]
            bi2, hwi2 = bi[flips], hwi[flips]
            zr = z.reshape(B, D, H * W)[bi2, :, hwi2]              # (F2, 512)
            cq = codebook[newidx]                                  # (F2, 512)
            ste = (zr + (cq - zr).astype(np.float32)).astype(np.float32)
            zq[bi2, :, hwi2] = ste
            idx[rowsel] = newidx

    zq = zq.reshape(B, D, H, W)

    # q_loss = 1.25 * mean((z_q - z_flat)^2) = 1.25 * mean(dist_min).
    # f32r error ~2e-5 on values ~512 is ~4e-8 relative — far below tolerance;
    # flipped rows change dist_min by < MARGIN, also negligible.
    c_loss = -np.sum(m1) / (B * H * W * D)
    q_loss = np.float32(1.25 * c_loss)

    return zq, q_loss, probs
